# revision 31
# baseline (speedup 1.0000x reference)
"""Causal self-attention kernel for Trainium2, 8-core SPMD.

Problem: B=4, T=2048, C=1024, H=16, D=64 (fp32).

Sharding: core = (batch b, head-group hg): 4 batches x 2 groups of 8 heads.
Each core computes attention for its 8 heads on its batch and a partial
out-projection; host sums the two partials per batch.

Per-core pipeline (layouts chosen so the only transpose is x -> x^T):
  A) x [T,C] -> x^T [C,T] via PE transposes (f32 -> f32r).
  B) QKV projection:
       qk^T[c_out, t]: lhsT=w_qkv chunk, rhs=x^T chunk
       v[t, c_out]:    lhsT=x^T chunk,  rhs=w_v chunk
  C) Per (q-block, head-pair): S^T[k,q] = (lhsT=K^T, rhs=Q^T), contraction
     d=64 with the two heads of a pair on array row-strips 0-1/2-3
     (base partitions 0/64) so their matmuls overlap in the PE array.
     One 1024-wide exp covers both heads' tiles; causal mask multiply;
     AV accumulates over k-tiles with lhsT=v_ext[k,65] whose 65th column of
     ones yields the softmax denominator as PSUM row 64. Normalize via
     gpsimd partition-broadcast + DVE reciprocal_approx_fast + multiply.
  D) y[t, c] partial = (lhsT=o^T chunk, rhs=w_out chunk) + b_out (hg==0).

Phases are emitted interleaved (A0 B0 | C(qb0,qb1) interleaved with A1 B1 |
C(qb2,qb3) interleaved with D) to keep the PE stream dense so the HAM clock
gate stays warm.

Matmuls run in float32r (fp32 with mantissa rounded to 11 bits, 1 cycle/row
at N>=256) for x/Q/K/scores/out-proj, and bf16 for P(=exp S)/V (the softmax
ratio cancels most of the P rounding error).
"""
import numpy as np
from contextlib import ExitStack

import concourse.bass as bass
import concourse.tile as tile
from concourse import bacc, mybir
from concourse import bass_utils
from concourse.masks import make_identity
import ml_dtypes

f32 = mybir.dt.float32
f32r = mybir.dt.float32r
f16 = mybir.dt.float16
bf16 = mybir.dt.bfloat16

B, T, C, H, D = 4, 2048, 1024, 16, 64
P = 128
NT = T // P            # 16 t-tiles
NCH = C // P           # 8 contraction chunks
QB = 512               # q block
NQB = T // QB          # 4
NH = 8                 # heads per core
VW = NH * 65           # 520: per-head 64 v cols + 1 ones col
N_CORES = 8

_EXP = mybir.ActivationFunctionType.Exp


def round_fp32r(x: np.ndarray) -> np.ndarray:
    """Round-to-nearest-even fp32 mantissa to 11 bits (fp32r), matching walrus."""
    bits = np.ascontiguousarray(x, np.float32).view(np.uint32)
    r = (bits + np.uint32(0x7FF) + ((bits >> np.uint32(12)) & np.uint32(1))) \
        & np.uint32(0xFFFFF000)
    return r.view(np.float32)


class _Ctx:
    pass


def _phase_a(g, nc, quarter):
    """Load + PE-transpose 4 t-tiles of one quarter into g.xT, with the
    per-tile V projections staggered two tiles behind. Yields per t-tile."""
    for ti in range(6):
        if ti < 4:
            tt = quarter * 4 + ti
            xin = g.xin_pool.tile([P, C], f32)
            nc.sync.dma_start(xin[:], g.x_d[tt * P:(tt + 1) * P, :])
            if quarter == 0 and ti == 0:
                for ch in range(NCH):
                    nc.sync.dma_start(g.wv_sb[:, ch * 512:(ch + 1) * 512],
                                      g.wv_d[ch * P:(ch + 1) * P, :])
                nc.sync.dma_start(g.masks_sb[:], g.masks_d[:])
            xc = g.xc_pool.tile([P, C], f16)
            if quarter == 0:
                nc.vector.tensor_copy(xc[:], xin[:])
            else:
                nc.scalar.activation(xc[:], xin[:], mybir.ActivationFunctionType.Copy)
            for cg in range(2):
                pst = g.ab_ps.tile([P, 4 * P], f16, tag="ab")
                for c2 in range(4):
                    ch = cg * 4 + c2
                    nc.tensor.transpose(pst[:, c2 * P:(c2 + 1) * P],
                                        xc[:, ch * P:(ch + 1) * P], g.ident[:])
                # strided copy: block c2 -> xT[:, (cg*4+c2)*QB + ti*128 ...]
                dst = g.xT[:, cg * 4 * QB:(cg + 1) * 4 * QB] \
                    .rearrange("p (c q) -> p c q", c=4)[:, :, ti * P:(ti + 1) * P]
                nc.vector.tensor_copy(dst, pst[:].rearrange("p (c q) -> p c q", c=4))
        if ti >= 2:
            _v_tile(g, nc, quarter, ti - 2)
        yield


def _phase_b(g, nc, quarter):
    """QK projection m-tiles for one t-quarter (= q-block). Yields per unit."""
    qb = quarter
    for m in range(8):
        # stream this m-tile's weights: [1024, 128] m-major block
        wqm = g.wqk_pool.tile([P, NCH * P], f16)
        src = g.wqk_d[m * C:(m + 1) * C, :].rearrange("(c p) j -> p c j", p=P)
        nc.sync.dma_start(wqm[:].rearrange("p (c j) -> p c j", c=NCH), src)
        ps = g.ab_ps.tile([P, QB], f32, tag="ab")
        for ch in range(NCH):
            nc.tensor.matmul(
                ps[:],
                wqm[:, ch * P:(ch + 1) * P],
                g.xT[:, ch * QB:(ch + 1) * QB],
                start=(ch == 0), stop=(ch == NCH - 1))
        nc.vector.tensor_scalar_add(
            g.qkT[:, m * T + qb * QB: m * T + (qb + 1) * QB],
            ps[:], g.bqk_sb[:, m:m + 1])
        yield


def _v_tile(g, nc, quarter, ti):
    """V projection for one t-tile (needs only that tile's x^T columns)."""
    tt = quarter * 4 + ti
    ps = g.ab_ps.tile([P, 512], f32, tag="ab")
    for ch in range(NCH):
        nc.tensor.matmul(
            ps[:],
            g.xT[:, ch * QB + ti * P: ch * QB + (ti + 1) * P],
            g.wv_sb[:, ch * 512:(ch + 1) * 512],
            start=(ch == 0), stop=(ch == NCH - 1))
    # scatter the 8x64 contiguous psum into the 65-strided v_ext layout
    dst = g.vext[:, tt * VW:(tt + 1) * VW] \
        .rearrange("p (h c) -> p h c", h=NH)[:, :, 0:64]
    src = ps[:].rearrange("p (h c) -> p h c", h=NH)
    bias = g.bvb[:].rearrange("p (h c) -> p h c", h=NH)
    nc.vector.tensor_add(dst, src, bias)


def _phase_c(g, nc, qb):
    """Attention for one q-block, 4 head pairs. Yields per k-tile and per norm."""
    nkt = (qb + 1) * 4
    for pr in range(4):
        hA, hB = 2 * pr, 2 * pr + 1
        avA = g.av_ps.tile([65, QB], f32, tag="av")
        avB = g.av_ps.tile([65, QB], f32, tag="av")
        for kt in range(nkt):
            j0 = max(kt - 4 * qb, 0) * P  # first live q column (0 off-diagonal)
            sc = g.sc_ps.tile([P, 1024], f32, tag="sc")
            for u, (h, base) in enumerate(((hA, 0), (hB, 64))):
                nc.tensor.matmul(
                    sc[:, u * 512 + j0:(u + 1) * 512],
                    g.qkT[base:base + 64, (4 + pr) * T + kt * P:
                          (4 + pr) * T + (kt + 1) * P],
                    g.qkT[base:base + 64, pr * T + qb * QB + j0:
                          pr * T + (qb + 1) * QB],
                    start=True, stop=True)
            pt = g.pt_pool.tile([P, 1024], bf16)
            if kt < 4 * qb:
                nc.scalar.activation(pt[:], sc[:], _EXP)
            else:
                j = kt - 4 * qb
                ptv = pt[:].rearrange("p (u q) -> p u q", u=2)
                scv = sc[:].rearrange("p (u q) -> p u q", u=2)
                if j > 0:
                    nc.vector.memset(ptv[:, :, 0:j * P], 0.0)
                nc.scalar.activation(ptv[:, :, j * P:512], scv[:, :, j * P:512], _EXP)
                band = g.masks_sb[:, 0:P].rearrange("p (o f) -> p o f", o=1) \
                    .to_broadcast((P, 2, P))
                nc.vector.tensor_mul(ptv[:, :, j * P:(j + 1) * P],
                                     ptv[:, :, j * P:(j + 1) * P], band)
            for u, (h, av) in enumerate(((hA, avA), (hB, avB))):
                nc.tensor.matmul(
                    av[:, j0:],
                    g.vext[:, kt * VW + h * 65: kt * VW + (h + 1) * 65],
                    pt[:, u * 512 + j0:(u + 1) * 512],
                    start=(kt == 0), stop=(kt == nkt - 1))
            yield
        for h, av in ((hA, avA), (hB, avB)):
            base = (h % 2) * 64
            o_un = g.on_pool.tile([65, QB], f32, tag="o_un")
            nc.vector.tensor_copy(o_un[:], av[:])
            srow = g.on_pool.tile([1, QB], f32, tag="srow")
            nc.gpsimd.dma_start(srow[:], o_un[64:65, :])
            sumb = g.on_pool.tile([64, QB], f32, tag="sumb")
            nc.gpsimd.partition_broadcast(sumb[:], srow[:])
            rcb = g.on_pool.tile([64, QB], f32, tag="rcb")
            nc.vector.reciprocal_approx_fast(rcb[:], sumb[:])
            dst = g.oT[base:base + 64, pr * T + qb * QB: pr * T + (qb + 1) * QB]
            if base == 0:
                nc.vector.tensor_mul(dst, o_un[0:64, :], rcb[:])
            else:
                # DVE cannot shift partitions; stage then DMA to rows 64..127
                ost = g.os_pool.tile([64, QB], f16)
                nc.vector.tensor_mul(ost[:], o_un[0:64, :], rcb[:])
                nc.gpsimd.dma_start(dst, ost[:])
            yield


def _phase_d(g, nc, tt):
    """Out-projection for one t-tile (both c blocks). Yields per cb."""
    for cb in range(2):
        ps = g.y_ps.tile([P, 512], f32)
        for j in range(4):
            nc.tensor.matmul(
                ps[:],
                g.oT[:, j * T + tt * P: j * T + (tt + 1) * P],
                g.wo_sb[:, j * C + cb * 512: j * C + (cb + 1) * 512],
                start=(j == 0), stop=(j == 3))
        ysb = g.ysb_pool.tile([P, 512], f32)
        nc.vector.tensor_add(ysb[:], ps[:], g.bob[:, cb * 512:(cb + 1) * 512])
        nc.sync.dma_start(
            g.y_d[tt * P:(tt + 1) * P, cb * 512:(cb + 1) * 512], ysb[:])
        yield


def _interleave(*gens):
    """Round-robin drain generators (weighted by remaining length implicitly)."""
    gens = [g for g in gens]
    while gens:
        done = []
        for g in gens:
            try:
                next(g)
            except StopIteration:
                done.append(g)
        for g in done:
            gens.remove(g)


def _chain(*gens):
    for g in gens:
        for _ in g:
            yield


def _trace(nc, debug=False):
    g = _Ctx()
    g.x_d = nc.dram_tensor("x", [T, C], f32, kind="ExternalInput").ap()
    # m-major layout: block m is the [C, 128] column slice for m-tile m
    g.wqk_d = nc.dram_tensor("wqk", [8 * C, P], f16, kind="ExternalInput").ap()
    wv_d = nc.dram_tensor("wv", [C, 512], f16, kind="ExternalInput").ap()
    wo_d = nc.dram_tensor("wo", [512, C], f16, kind="ExternalInput").ap()
    bqk_d = nc.dram_tensor("bqk", [P, 8], f32, kind="ExternalInput").ap()
    bv_d = nc.dram_tensor("bv", [1, 512], f32, kind="ExternalInput").ap()
    bo_d = nc.dram_tensor("bo", [1, C], f32, kind="ExternalInput").ap()
    masks_d = nc.dram_tensor("masks", [P, 4096], bf16, kind="ExternalInput").ap()
    g.y_d = nc.dram_tensor("y", [T, C], f32, kind="ExternalOutput").ap()
    if debug:
        d_qkT = nc.dram_tensor("d_qkT", [P, 8 * T], f16, kind="ExternalOutput").ap()
        d_vext = nc.dram_tensor("d_vext", [P, NT * VW], bf16, kind="ExternalOutput").ap()
        d_oT = nc.dram_tensor("d_oT", [P, 4 * T], f16, kind="ExternalOutput").ap()

    with tile.TileContext(nc) as tc, ExitStack() as octx:
        pers = octx.enter_context(tc.tile_pool(name="pers", bufs=1))
        g.qkT = pers.tile([P, 8 * T], f16)
        g.vext = pers.tile([P, NT * VW], bf16)
        g.bqk_sb = pers.tile([P, 8], f32)
        g.bvb = pers.tile([P, 512], f32)
        g.ident = pers.tile([P, P], f16)
        nc.sync.dma_start(g.bqk_sb[:], bqk_d[:])
        bv_row = pers.tile([1, 512], f32)
        nc.sync.dma_start(bv_row[:], bv_d[:])
        nc.gpsimd.partition_broadcast(g.bvb[:], bv_row[:])
        make_identity(nc, g.ident[:])
        # ones columns of v_ext (65th col per head), once for all 16 k-tiles
        ones_cols = g.vext[:].rearrange("p (t h c) -> p t h c", t=NT, h=NH)[:, :, :, 64:65]
        nc.vector.memset(ones_cols, 1.0)

        with ExitStack() as cdctx:
            wcd = cdctx.enter_context(tc.tile_pool(name="wcd", bufs=1))
            g.oT = wcd.tile([P, 4 * T], f16)
            g.masks_sb = wcd.tile([P, 4096], bf16)
            g.masks_d = masks_d

            g.sc_ps = cdctx.enter_context(tc.tile_pool(name="sc_ps", bufs=2, space="PSUM"))
            g.av_ps = cdctx.enter_context(tc.tile_pool(name="av_ps", bufs=2, space="PSUM"))
            g.pt_pool = cdctx.enter_context(tc.tile_pool(name="pt", bufs=4))
            g.on_pool = cdctx.enter_context(tc.tile_pool(name="onorm", bufs=3))
            g.os_pool = cdctx.enter_context(tc.tile_pool(name="ostage", bufs=3))

            with ExitStack() as abctx:
                wab = abctx.enter_context(tc.tile_pool(name="wab", bufs=1))
                g.wv_sb = wab.tile([P, 8 * 512], f16)
                g.wv_d = wv_d
                g.wqk_pool = abctx.enter_context(tc.tile_pool(name="wqk", bufs=3))
                g.xin_pool = abctx.enter_context(tc.tile_pool(name="xin", bufs=3))
                g.xc_pool = abctx.enter_context(tc.tile_pool(name="xc", bufs=3))
                xT_pool = abctx.enter_context(tc.tile_pool(name="xT", bufs=1))
                g.ab_ps = abctx.enter_context(
                    tc.tile_pool(name="ab_ps", bufs=2, space="PSUM"))
                g.xT = xT_pool.tile([P, NCH * QB], f16)

                # quarter 0: A, B sequential; then C(qb) overlapped with
                # the next quarter's A/B
                for _ in _chain(_phase_a(g, nc, 0), _phase_b(g, nc, 0)):
                    pass
                for q in (1, 2, 3):
                    _interleave(
                        _phase_c(g, nc, q - 1),
                        _chain(_phase_a(g, nc, q), _phase_b(g, nc, q)),
                    )

            # phase D pools (ab pools closed above free SBUF + PSUM banks)
            wd = cdctx.enter_context(tc.tile_pool(name="wd", bufs=1))
            g.wo_sb = wd.tile([P, 4 * C], f16)
            for j in range(4):
                nc.sync.dma_start(g.wo_sb[:, j * C:(j + 1) * C],
                                  wo_d[j * P:(j + 1) * P, :])
            g.bob = wd.tile([P, C], f32)
            bo_row = wd.tile([1, C], f32)
            nc.sync.dma_start(bo_row[:], bo_d[:])
            nc.gpsimd.partition_broadcast(g.bob[:], bo_row[:])
            g.y_ps = cdctx.enter_context(tc.tile_pool(name="y_ps", bufs=2, space="PSUM"))
            g.ysb_pool = cdctx.enter_context(tc.tile_pool(name="ysb", bufs=3))
            # C(qb3) + D(tt 0..11), then D(tt 12..15)
            _interleave(
                _phase_c(g, nc, 3),
                _chain(*[_phase_d(g, nc, tt) for tt in range(0, 12)]),
            )
            for _ in _chain(*[_phase_d(g, nc, tt) for tt in range(12, 16)]):
                pass

            if debug:
                nc.sync.dma_start(d_qkT[:], g.qkT[:])
                nc.sync.dma_start(d_vext[:], g.vext[:])
                nc.sync.dma_start(d_oT[:], g.oT[:])


_nc_cache = {}


def _build(debug=False):
    if debug not in _nc_cache:
        nc = bacc.Bacc("TRN2", target_bir_lowering=False, debug=False,
                       num_devices=N_CORES)
        _trace(nc, debug=debug)
        nc.compile()
        _nc_cache[debug] = nc
    return _nc_cache[debug]


def make_in_maps(x, w_qkv, b_qkv, w_out, b_out):
    x = np.asarray(x, np.float32)
    w_qkv = np.asarray(w_qkv, np.float32)
    b_qkv = np.asarray(b_qkv, np.float32)
    w_out = np.asarray(w_out, np.float32)
    b_out = np.asarray(b_out, np.float32)

    # causal mask tiles, doubled for the two-head batch:
    # masks[:, j*1024 + u*512 + f] = (f >= p + 128*j) for u in (0, 1)
    pidx = np.arange(P)[:, None]
    fidx = np.arange(512)[None, :]
    masks = np.zeros((P, 4096), np.float32)
    for j in range(4):
        m = (fidx >= pidx + P * j)
        masks[:, j * 1024:j * 1024 + 512] = m
        masks[:, j * 1024 + 512:(j + 1) * 1024] = m
    masks = masks.astype(ml_dtypes.bfloat16)

    scale = 1.0 / np.sqrt(D)
    in_maps = []
    for core in range(N_CORES):
        b_i, hg = core // 2, core % 2
        sl = slice(hg * 512, (hg + 1) * 512)
        wq = w_qkv[:, 0:1024][:, sl] * scale
        wk = w_qkv[:, 1024:2048][:, sl]
        wv = w_qkv[:, 2048:3072][:, sl]
        bq = b_qkv[0:1024][sl] * scale
        bk = b_qkv[1024:2048][sl]
        bv = b_qkv[2048:3072][sl]

        wqk = np.concatenate([wq, wk], axis=1)              # [1024, 1024]
        # m-major: [8, 1024, 128] flattened
        wqk = wqk.reshape(C, 8, P).transpose(1, 0, 2).reshape(8 * C, P)
        bqk = np.concatenate([bq, bk]).reshape(8, P).T.copy()  # [128, 8]
        wo = w_out[sl, :]
        bo = b_out[None, :] if hg == 0 else np.zeros((1, C), np.float32)

        in_maps.append({
            "x": np.ascontiguousarray(x[b_i]),
            "wqk": wqk.astype(np.float16),
            "wv": np.ascontiguousarray(wv).astype(np.float16),
            "wo": wo.astype(np.float16),
            "bqk": np.ascontiguousarray(bqk),
            "bv": np.ascontiguousarray(bv[None, :]),
            "bo": np.ascontiguousarray(bo),
            "masks": masks,
        })
    return in_maps


def run(in_maps, debug=False, **kwargs):
    nc = _build(debug=debug)
    return bass_utils.run_bass_kernel_spmd(nc, in_maps,
                                           core_ids=list(range(N_CORES)), **kwargs)


def kernel(x, w_qkv, b_qkv, w_out, b_out):
    in_maps = make_in_maps(x, w_qkv, b_qkv, w_out, b_out)
    res = run(in_maps)
    y = np.zeros((B, T, C), np.float32)
    for core in range(N_CORES):
        y[core // 2] += res.results[core]["y"]
    return y
